# revision 1
# baseline (speedup 1.0000x reference)
"""KANConv kernel for Trainium2 (8 NeuronCores, data-parallel over batch).

Math: out = conv2d_same(x, spline_weights.sum(-1)) + conv2d_same(silu(x), basis_weights)
    == conv2d_same(concat([x, silu(x)], ch), concat([w_spline, w_basis], cin))

Device strategy (per core, 2 images):
  - Host zero-pads x spatially to (130, 130); host folds spline G-sum and
    concatenates weights to a single (9, 128cin, 128cout) tap-major tensor.
  - SBUF tile [128p, 18, 130]: partitions 0..63 hold x rows r0-1..r0+16,
    partitions 64..127 hold silu(x) (computed on ScalarE).
  - Conv = 9 shifted matmuls accumulating in PSUM: for each 4-output-row
    block, psum[cout, 512] += w_tap[cin, cout].T @ x_shift[cin, 512].
  - float32r matmuls: fp32 in/out at full PE streaming rate for N=512.
"""

import numpy as np

import concourse.bass as bass
from concourse import bacc
import concourse.mybir as mybir
import concourse.tile as tile
from concourse.bass_utils import run_bass_kernel_spmd

B, CIN, COUT, H, W = 16, 64, 128, 128, 128
KH = KW = 3
G = 4
N_CORES = 8
B_LOC = B // N_CORES  # 2 images per core

P = 128           # partitions (= concat channel dim = cout)
HP, WP = H + 2, W + 2
STRIP = 16        # output rows per strip
ROWS_IN = STRIP + 2
NSTRIPS = H // STRIP
FREE = 512        # psum free dim (fp32 bank)
RPM = FREE // W   # output rows per matmul = 4
NPS = STRIP // RPM  # psum tiles per strip = 4


def build_conv(tc, out_ap, xp_ap, w_ap, reps=1, loop_n=0):
    nc = tc.nc
    f32 = mybir.dt.float32
    f32r = mybir.dt.float32r
    import contextlib

    with (
        tc.tile_pool(name="wpool", bufs=1) as wpool,
        tc.tile_pool(name="xpool", bufs=3) as xpool,
        tc.tile_pool(name="opool", bufs=3) as opool,
        tc.tile_pool(name="psum", bufs=2, space="PSUM") as psum_pool,
    ):
        # weights DRAM (9, 128cin, 128cout) -> SBUF [cin=128, 9, cout=128]
        wt = wpool.tile([P, KH * KW, COUT], f32r)
        nc.sync.dma_start(out=wt[:], in_=w_ap.rearrange("t k m -> k t m"))
        loop_ctx = tc.For_i(0, loop_n, 1) if loop_n else contextlib.nullcontext()
        with loop_ctx:
            for img, s in [
                (i, j)
                for _ in range(reps)
                for i in range(B_LOC)
                for j in range(NSTRIPS)
            ]:
                r0 = s * STRIP  # padded-row index of first halo row
                xt = xpool.tile([P, ROWS_IN, WP], f32r)
                nc.sync.dma_start(
                    out=xt[:CIN], in_=xp_ap[img, :, r0 : r0 + ROWS_IN, :]
                )
                # silu(x) into upper 64 partitions (silu(0)=0 keeps padding valid)
                nc.scalar.activation(
                    out=xt[CIN:], in_=xt[:CIN],
                    func=mybir.ActivationFunctionType.Silu,
                )
                ot = opool.tile([P, STRIP, W], f32)
                for j in range(NPS):
                    pt = psum_pool.tile([P, FREE], f32, name=f"ps{j}")
                    for t in range(KH * KW):
                        dh, dw = t // KW, t % KW
                        rhs = xt[:, RPM * j + dh : RPM * j + dh + RPM, dw : dw + W]
                        nc.tensor.matmul(
                            pt[:],
                            wt[:, t, :],
                            rhs,
                            start=(t == 0),
                            stop=(t == KH * KW - 1),
                        )
                    nc.vector.tensor_copy(
                        out=ot[:, RPM * j : RPM * (j + 1), :],
                        in_=pt[:].rearrange("p (r w) -> p r w", w=W),
                    )
                nc.sync.dma_start(
                    out=out_ap[img, :, s * STRIP : (s + 1) * STRIP, :], in_=ot[:]
                )


_CACHE = {}


def _get_nc(reps=1, loop_n=0, bench_io=False):
    key = ("nc", reps, loop_n, bench_io)
    if key not in _CACHE:
        nc = bacc.Bacc("TRN2", target_bir_lowering=False, debug=False)
        if bench_io:
            # timing-only variant: conv runs on internal DRAM (garbage data),
            # external I/O is tiny so tunnel transfer cost ~0
            dummy_in = nc.dram_tensor(
                "dummy_in", [1, 4], mybir.dt.float32, kind="ExternalInput"
            ).ap()
            dummy_out = nc.dram_tensor(
                "dummy_out", [1, 4], mybir.dt.float32, kind="ExternalOutput"
            ).ap()
            xp = nc.dram_tensor("xp", [B_LOC, CIN, HP, WP], mybir.dt.float32r).ap()
            w = nc.dram_tensor("w", [KH * KW, P, COUT], mybir.dt.float32r).ap()
            out = nc.dram_tensor("out", [B_LOC, COUT, H, W], mybir.dt.float32).ap()
        else:
            xp = nc.dram_tensor(
                "xp", [B_LOC, CIN, HP, WP], mybir.dt.float32r, kind="ExternalInput"
            ).ap()
            w = nc.dram_tensor(
                "w", [KH * KW, P, COUT], mybir.dt.float32r, kind="ExternalInput"
            ).ap()
            out = nc.dram_tensor(
                "out", [B_LOC, COUT, H, W], mybir.dt.float32, kind="ExternalOutput"
            ).ap()
        with tile.TileContext(nc) as tc:
            if bench_io:
                with tc.tile_pool(name="dummy", bufs=1) as dpool:
                    dt_ = dpool.tile([1, 4], mybir.dt.float32)
                    nc.sync.dma_start(out=dt_[:], in_=dummy_in)
                    build_conv(tc, out, xp, w, reps=reps, loop_n=loop_n)
                    nc.sync.dma_start(out=dummy_out, in_=dt_[:])
            else:
                build_conv(tc, out, xp, w, reps=reps, loop_n=loop_n)
        nc.finalize()
        _CACHE[key] = nc
    return _CACHE[key]


def _round_fp32r(a):
    # RTNE fp32 -> e8m11 (low 12 mantissa bits zero), the FP32R encoding.
    u = np.ascontiguousarray(a, dtype=np.float32).view(np.uint32)
    lsb = (u >> 12) & np.uint32(1)
    r = (u + np.uint32(0x7FF) + lsb) & np.uint32(0xFFFFF000)
    return r.view(np.float32)


def _prep_inputs(x, spline_weights, basis_weights):
    w_spline = spline_weights.sum(axis=-1)  # (COUT, CIN, 3, 3)
    w_cat = np.concatenate([w_spline, basis_weights], axis=1)  # (COUT, 128, 3, 3)
    # -> (tap, cin_cat, cout)
    w_taps = _round_fp32r(
        np.ascontiguousarray(w_cat.transpose(2, 3, 1, 0).reshape(KH * KW, P, COUT))
    )
    x_pad = _round_fp32r(np.pad(x, ((0, 0), (0, 0), (1, 1), (1, 1))))
    return x_pad, w_taps


def kernel(x, spline_weights, basis_weights, _trace=False, _tmpdir=None):
    x = np.asarray(x, dtype=np.float32)
    spline_weights = np.asarray(spline_weights, dtype=np.float32)
    basis_weights = np.asarray(basis_weights, dtype=np.float32)
    x_pad, w_taps = _prep_inputs(x, spline_weights, basis_weights)
    nc = _get_nc()
    in_maps = [
        {"xp": x_pad[B_LOC * c : B_LOC * (c + 1)], "w": w_taps}
        for c in range(N_CORES)
    ]
    res = run_bass_kernel_spmd(
        nc, in_maps, list(range(N_CORES)), trace=_trace, tmpdir=_tmpdir
    )
    out = np.concatenate([res.results[c]["out"] for c in range(N_CORES)], axis=0)
    if _trace:
        kernel.last_results = res
    return out



# revision 2
# speedup vs baseline: 1.1242x; 1.1242x over previous
"""KANConv kernel for Trainium2 (8 NeuronCores, data-parallel over batch).

Math: out = conv2d_same(x, spline_weights.sum(-1)) + conv2d_same(silu(x), basis_weights)
    == conv2d_same(concat([x, silu(x)], ch), concat([w_spline, w_basis], cin))

Device strategy (per core, 2 images):
  - Host zero-pads x spatially to (130, 130), casts to fp16; host folds the
    spline G-sum and concatenates weights to a single (128cin, 9tap, 128cout)
    fp16 tensor.
  - Whole padded image resident in SBUF: tile [128p, 130, 130] fp16
    (33.8KB/partition), partitions 0..63 = x, 64..127 = silu(x) (ScalarE;
    silu(0)=0 keeps the zero padding valid). Loaded in row chunks so the
    matmul stream starts as soon as the first rows + tap weights land.
  - Conv = 9 shifted matmuls accumulating in PSUM: per 4-output-row block j,
    psum[cout, 512] += w_tap[cin, cout].T @ x_shift[cin, 512]. fp16 operands
    stream at 1 row/cycle (same as fp32r) but LDWEIGHTS is fast-weight-load
    eligible, hiding the per-matmul weight load behind the previous stream.
  - PSUM: 8 banks rotating; DVE copies each bank to an SBUF staging tile;
    output DMAd per 8-row block from GpSimd to keep the exposed tail small.
"""

import numpy as np

import concourse.bass as bass
from concourse import bacc
import concourse.mybir as mybir
import concourse.tile as tile
from concourse.bass_utils import run_bass_kernel_spmd

B, CIN, COUT, H, W = 16, 64, 128, 128, 128
KH = KW = 3
G = 4
N_CORES = 8
B_LOC = B // N_CORES  # 2 images per core

P = 128           # partitions (= concat channel dim = cout)
HP, WP = H + 2, W + 2
FREE = 512        # psum free dim (fp32 bank)
RPM = FREE // W   # output rows per matmul/psum block = 4
NJ = H // RPM     # psum blocks per image = 32
JPD = 2           # psum blocks per output DMA (8 rows, 4KB/partition)

# x row chunks: small first chunk so the matmul stream starts early
ROWS = [(0, 16), (16, 32), (32, 64), (64, 96), (96, HP)]


def build_conv(tc, out_ap, xp_ap, w_ap):
    nc = tc.nc
    f16 = mybir.dt.float16
    f32 = mybir.dt.float32

    with (
        tc.tile_pool(name="wpool", bufs=1) as wpool,
        tc.tile_pool(name="xpool", bufs=2) as xpool,
        tc.tile_pool(name="opool", bufs=4) as opool,
        tc.tile_pool(name="psum", bufs=8, space="PSUM") as psum_pool,
    ):
        wt = None
        for img in range(B_LOC):
            xt = xpool.tile([P, HP, WP], f16)
            for r0, r1 in ROWS:
                nc.sync.dma_start(
                    out=xt[:CIN, r0:r1], in_=xp_ap[img, :, r0:r1, :]
                )
                if wt is None:
                    # weights issued right after the first x chunk
                    wt = wpool.tile([P, KH * KW, COUT], f16)
                    nc.sync.dma_start(out=wt[:], in_=w_ap)
                nc.scalar.activation(
                    out=xt[CIN:, r0:r1], in_=xt[:CIN, r0:r1],
                    func=mybir.ActivationFunctionType.Silu,
                )
            for jj in range(0, NJ, JPD):
                ot = opool.tile([P, JPD * RPM, W], f32)
                for j in range(jj, jj + JPD):
                    pt = psum_pool.tile([P, FREE], f32, name="ps")
                    for t in range(KH * KW):
                        dh, dw = t // KW, t % KW
                        rhs = xt[:, RPM * j + dh : RPM * j + dh + RPM, dw : dw + W]
                        nc.tensor.matmul(
                            pt[:],
                            wt[:, t, :],
                            rhs,
                            start=(t == 0),
                            stop=(t == KH * KW - 1),
                        )
                    nc.vector.tensor_copy(
                        out=ot[:, (j - jj) * RPM : (j - jj + 1) * RPM, :],
                        in_=pt[:].rearrange("p (r w) -> p r w", w=W),
                    )
                nc.gpsimd.dma_start(
                    out=out_ap[img, :, jj * RPM : (jj + JPD) * RPM, :], in_=ot[:]
                )


_CACHE = {}


def _get_nc():
    key = "nc"
    if key not in _CACHE:
        nc = bacc.Bacc("TRN2", target_bir_lowering=False, debug=False)
        xp = nc.dram_tensor(
            "xp", [B_LOC, CIN, HP, WP], mybir.dt.float16, kind="ExternalInput"
        ).ap()
        w = nc.dram_tensor(
            "w", [P, KH * KW, COUT], mybir.dt.float16, kind="ExternalInput"
        ).ap()
        out = nc.dram_tensor(
            "out", [B_LOC, COUT, H, W], mybir.dt.float32, kind="ExternalOutput"
        ).ap()
        with tile.TileContext(nc) as tc:
            build_conv(tc, out, xp, w)
        nc.finalize()
        _CACHE[key] = nc
    return _CACHE[key]


def _prep_inputs(x, spline_weights, basis_weights):
    w_spline = spline_weights.sum(axis=-1)  # (COUT, CIN, 3, 3)
    w_cat = np.concatenate([w_spline, basis_weights], axis=1)  # (COUT, 128, 3, 3)
    # -> (cin_cat, tap, cout)
    w_ktm = np.ascontiguousarray(
        w_cat.transpose(1, 2, 3, 0).reshape(P, KH * KW, COUT).astype(np.float16)
    )
    x_pad = np.pad(x, ((0, 0), (0, 0), (1, 1), (1, 1))).astype(np.float16)
    return x_pad, w_ktm


def kernel(x, spline_weights, basis_weights, _trace=False, _tmpdir=None):
    x = np.asarray(x, dtype=np.float32)
    spline_weights = np.asarray(spline_weights, dtype=np.float32)
    basis_weights = np.asarray(basis_weights, dtype=np.float32)
    x_pad, w_ktm = _prep_inputs(x, spline_weights, basis_weights)
    nc = _get_nc()
    in_maps = [
        {"xp": x_pad[B_LOC * c : B_LOC * (c + 1)], "w": w_ktm}
        for c in range(N_CORES)
    ]
    res = run_bass_kernel_spmd(
        nc, in_maps, list(range(N_CORES)), trace=_trace, tmpdir=_tmpdir
    )
    out = np.concatenate([res.results[c]["out"] for c in range(N_CORES)], axis=0)
    if _trace:
        kernel.last_results = res
    return out


# revision 6
# speedup vs baseline: 1.1440x; 1.0176x over previous
"""KANConv kernel for Trainium2 (8 NeuronCores, data-parallel over batch).

Math: out = conv2d_same(x, spline_weights.sum(-1)) + conv2d_same(silu(x), basis_weights)
    == conv2d_same(concat([x, silu(x)], ch), concat([w_spline, w_basis], cin))

Device strategy (per core, 2 images):
  - Host zero-pads x spatially to (130, 130), casts to fp16; host folds the
    spline G-sum and concatenates weights to a single (128cin, 9tap, 128cout)
    fp16 tensor.
  - Whole padded image resident in SBUF: tile [128p, 130, 130] fp16
    (33.8KB/partition), partitions 0..63 = x, 64..127 = silu(x) (ScalarE;
    silu(0)=0 keeps the zero padding valid). Loaded in row chunks so the
    matmul stream starts as soon as the first rows + tap weights land.
  - Conv = 9 shifted matmuls accumulating in PSUM: per 4-output-row block j,
    psum[cout, 512] += w_tap[cin, cout].T @ x_shift[cin, 512]. fp16 operands
    stream at 1 row/cycle (same as fp32r) but LDWEIGHTS is fast-weight-load
    eligible, hiding the per-matmul weight load behind the previous stream.
  - PSUM: 8 banks rotating; DVE copies each bank to an SBUF staging tile;
    output DMAd per 8-row block from GpSimd to keep the exposed tail small.
"""

import numpy as np

import concourse.bass as bass
from concourse import bacc
import concourse.mybir as mybir
import concourse.tile as tile
from concourse.bass_utils import run_bass_kernel_spmd

B, CIN, COUT, H, W = 16, 64, 128, 128, 128
KH = KW = 3
G = 4
N_CORES = 8
B_LOC = B // N_CORES  # 2 images per core

P = 128           # partitions (= concat channel dim = cout)
HP, WP = H + 2, W + 2
FREE = 512        # psum free dim (fp32 bank)
RPM = FREE // W   # output rows per matmul/psum block = 4
NJ = H // RPM     # psum blocks per image = 32
JPD = 2           # psum blocks per output DMA (8 rows, 4KB/partition)

# x row chunks: tiny first chunk so the matmul stream starts early
ROWS0 = [(0, 6), (6, 16), (16, 32), (32, 64), (64, 96), (96, HP)]
ROWS1 = [(0, 32), (32, 64), (64, 96), (96, HP)]
N_WARM = 8  # PE p-state warmup matmuls during the prologue


def build_conv(tc, out_ap, xp_ap, w_ap, sink_ap):
    nc = tc.nc
    f16 = mybir.dt.float16
    f32 = mybir.dt.float32

    with (
        tc.tile_pool(name="wpool", bufs=1) as wpool,
        tc.tile_pool(name="xpool", bufs=2) as xpool,
        tc.tile_pool(name="opool", bufs=4) as opool,
        tc.tile_pool(name="psum", bufs=5, space="PSUM") as psum_pool,
        tc.tile_pool(name="psum_fin", bufs=1, space="PSUM") as psum_fin,
    ):
        # PE warmup: dummy K=1/M=1 matmuls streaming N=512 rows ramp the
        # tensor engine to full clock while the first x chunk + silu land.
        # The psum result is sunk to DRAM so the chain is observably live.
        wscr = wpool.tile([1, FREE], f16, name="warm_src")
        nc.gpsimd.memset(wscr[:], 0.0)
        pt_w = psum_pool.tile([P, FREE], f32, name="ps")
        for _ in range(N_WARM):
            nc.tensor.matmul(pt_w[:1, :], wscr[:, :1], wscr[:], start=True, stop=True)
        wsink = wpool.tile([1, 4], f32, name="warm_sink")
        nc.vector.tensor_copy(out=wsink[:], in_=pt_w[:1, :4])
        nc.gpsimd.dma_start(out=sink_ap, in_=wsink[:])

        wt = None
        for img in range(B_LOC):
            xt = xpool.tile([P, HP, WP], f16)
            for r0, r1 in ROWS0 if img == 0 else ROWS1:
                nc.sync.dma_start(
                    out=xt[:CIN, r0:r1], in_=xp_ap[img, :, r0:r1, :]
                )
                if wt is None:
                    # weights issued right after the first x chunk
                    wt = wpool.tile([P, KH * KW, COUT], f16)
                    nc.sync.dma_start(out=wt[:], in_=w_ap)
                nc.scalar.activation(
                    out=xt[CIN:, r0:r1], in_=xt[:CIN, r0:r1],
                    func=mybir.ActivationFunctionType.Silu,
                )
            last = img == B_LOC - 1
            for jj in range(0, NJ, JPD):
                if last and jj == NJ - JPD:
                    break
                ot = opool.tile([P, JPD * RPM, W], f32)
                for j in range(jj, jj + JPD):
                    pt = psum_pool.tile([P, FREE], f32, name="ps")
                    for t in range(KH * KW):
                        dh, dw = t // KW, t % KW
                        rhs = xt[:, RPM * j + dh : RPM * j + dh + RPM, dw : dw + W]
                        nc.tensor.matmul(
                            pt[:],
                            wt[:, t, :],
                            rhs,
                            start=(t == 0),
                            stop=(t == KH * KW - 1),
                        )
                    nc.vector.tensor_copy(
                        out=ot[:, (j - jj) * RPM : (j - jj + 1) * RPM, :],
                        in_=pt[:].rearrange("p (r w) -> p r w", w=W),
                    )
                nc.gpsimd.dma_start(
                    out=out_ap[img, :, jj * RPM : (jj + JPD) * RPM, :], in_=ot[:]
                )
        # final 2 j-blocks in progressively smaller pieces so the exposed
        # copy + DMA after the very last matmul is minimal
        img = B_LOC - 1
        j = NJ - 2
        ot = opool.tile([P, RPM, W], f32, name="ot_fin")
        pt = psum_pool.tile([P, FREE], f32, name="ps")
        for t in range(KH * KW):
            dh, dw = t // KW, t % KW
            rhs = xt[:, RPM * j + dh : RPM * j + dh + RPM, dw : dw + W]
            nc.tensor.matmul(pt[:], wt[:, t, :], rhs, start=(t == 0), stop=(t == KH * KW - 1))
        nc.vector.tensor_copy(
            out=ot[:], in_=pt[:].rearrange("p (r w) -> p r w", w=W)
        )
        nc.gpsimd.dma_start(
            out=out_ap[img, :, RPM * j : RPM * (j + 1), :], in_=ot[:]
        )
        # very last 4 output rows as two 2-row half-bank blocks
        for half in range(2):
            r = H - RPM + 2 * half
            oth = opool.tile([P, 2, W], f32, name=f"ot_h{half}")
            pth = psum_fin.tile([P, 2 * W], f32, name=f"ps_h{half}")
            for t in range(KH * KW):
                dh, dw = t // KW, t % KW
                rhs = xt[:, r + dh : r + dh + 2, dw : dw + W]
                nc.tensor.matmul(pth[:], wt[:, t, :], rhs, start=(t == 0), stop=(t == KH * KW - 1))
            nc.vector.tensor_copy(
                out=oth[:], in_=pth[:].rearrange("p (r w) -> p r w", w=W)
            )
            nc.gpsimd.dma_start(out=out_ap[img, :, r : r + 2, :], in_=oth[:])


_CACHE = {}


def _get_nc():
    key = "nc"
    if key not in _CACHE:
        nc = bacc.Bacc("TRN2", target_bir_lowering=False, debug=False)
        xp = nc.dram_tensor(
            "xp", [B_LOC, CIN, HP, WP], mybir.dt.float16, kind="ExternalInput"
        ).ap()
        w = nc.dram_tensor(
            "w", [P, KH * KW, COUT], mybir.dt.float16, kind="ExternalInput"
        ).ap()
        out = nc.dram_tensor(
            "out", [B_LOC, COUT, H, W], mybir.dt.float32, kind="ExternalOutput"
        ).ap()
        sink = nc.dram_tensor("warm_sink", [1, 4], mybir.dt.float32).ap()
        with tile.TileContext(nc) as tc:
            build_conv(tc, out, xp, w, sink)
        nc.finalize()
        _CACHE[key] = nc
    return _CACHE[key]


def _prep_inputs(x, spline_weights, basis_weights):
    w_spline = spline_weights.sum(axis=-1)  # (COUT, CIN, 3, 3)
    w_cat = np.concatenate([w_spline, basis_weights], axis=1)  # (COUT, 128, 3, 3)
    # -> (cin_cat, tap, cout)
    w_ktm = np.ascontiguousarray(
        w_cat.transpose(1, 2, 3, 0).reshape(P, KH * KW, COUT).astype(np.float16)
    )
    x_pad = np.pad(x, ((0, 0), (0, 0), (1, 1), (1, 1))).astype(np.float16)
    return x_pad, w_ktm


def kernel(x, spline_weights, basis_weights, _trace=False, _tmpdir=None):
    x = np.asarray(x, dtype=np.float32)
    spline_weights = np.asarray(spline_weights, dtype=np.float32)
    basis_weights = np.asarray(basis_weights, dtype=np.float32)
    x_pad, w_ktm = _prep_inputs(x, spline_weights, basis_weights)
    nc = _get_nc()
    in_maps = [
        {"xp": x_pad[B_LOC * c : B_LOC * (c + 1)], "w": w_ktm}
        for c in range(N_CORES)
    ]
    res = run_bass_kernel_spmd(
        nc, in_maps, list(range(N_CORES)), trace=_trace, tmpdir=_tmpdir
    )
    out = np.concatenate([res.results[c]["out"] for c in range(N_CORES)], axis=0)
    if _trace:
        kernel.last_results = res
    return out
